# revision 51
# baseline (speedup 1.0000x reference)
"""Trainium2 Bass kernel for the ADMG RKHS-DAGMA gradient contraction.

Reference computation (D=8 variables, N=1500 observations):
    output[i, j] = sum_l alpha[j, l] * K[j, i, l]
                 + sum_{a, l} beta[j, a, l] * grad_K2[j, i, l, a]     [N, D]
    Sigma = L @ L.T + 1e-6 * I                                        [D, D]

Sharding: variable-parallel over the leading d axis — core j owns K[j]
(9 MB) and grad_K2[j] (72 MB) and produces output column j. No
collectives are needed; columns are gathered on the host.

Per-core kernel: a weighted row-sum. With G = grad_K2[j] viewed as
[N, L*A] = [1500, 12000] and vrow = concat(beta[j].T.flat, alpha[j])
(13500 f32), the output column is
    out[i] = sum_m [G | K][i, m] * vrow[m].
vrow is broadcast across the 128 partitions on-chip (PE ones-matmul into
PSUM, ScalarE drains to SBUF), then every streamed [128, C] tile goes
through one fused DVE scalar_tensor_tensor (multiply + free-axis
accumulate), so each element passes the VectorEngine exactly once
(~170us) and the kernel stays DMA-bound (~82 MB/core at ~360 GB/s).

TRN2 toolchain constraints baked in here:
  - every instruction except EventSemaphore encodes exactly ONE
    semaphore wait; Tile emits inline waits, so the kernel is structured
    (private acc tiles, clock-priming reads) to never need two;
  - the raw-ISA tensor_tensor_reduce encoding crashes the exec unit on
    this runtime; InstTensorScalarPtr (scalar_tensor_tensor with
    accum_out) performs the same fused multiply+reduce;
  - Sigma's +1e-6*I rides inside the matmul via an augmented-contraction
    operand [L.T ; 1e-3*I] so no PSUM+SBUF add is needed.
"""

import numpy as np

D = 8
N = 1500
NCORES = 8
MG = D * N            # 12000: grad_K2 inner (l, a) length
MTOT = MG + N         # 13500: plus K's l axis
P = 128
NT = (N + P - 1) // P  # 12 i-tiles (last one 92 rows)
BC = 512               # broadcast chunk: one fp32 matmul free dim / PSUM bank
NBC = (MTOT + BC - 1) // BC          # 27 chunks
MPAD = NBC * BC                      # 13824
VHLEN = MPAD + P                     # vrow (padded) + ones-vector tail
GC = MG // 2           # 6000-wide g chunks (two per i-tile)


_COMPILED = None


def _build():
    from concourse import bacc, mybir
    from concourse.tile import TileContext

    f32 = mybir.dt.float32
    nc = bacc.Bacc()

    g = nc.declare_dram_parameter("g", [N, MG], f32, isOutput=False)
    k = nc.declare_dram_parameter("k", [N, N], f32, isOutput=False)
    # One row: vrow zero-padded to MPAD, then 128 ones (the PE-broadcast
    # lhsT) — a single row so both matmul operands sit at base partition 0
    # and depend on one DMA.
    vh = nc.declare_dram_parameter("vh", [1, VHLEN], f32, isOutput=False)
    # Rows 0..D-1 = L.T, rows D..2D-1 = 1e-3 * I, so that one matmul gives
    # ltaug.T @ ltaug = L @ L.T + 1e-6 * I.
    lt = nc.declare_dram_parameter("lt", [2 * D, D], f32, isOutput=False)
    o = nc.declare_dram_parameter("o", [P, NT], f32, isOutput=True)
    sig = nc.declare_dram_parameter("sig", [D, D], f32, isOutput=True)

    mult = mybir.AluOpType.mult
    add = mybir.AluOpType.add

    with TileContext(nc) as tc:
        with (
            tc.tile_pool(name="const", bufs=1) as cpool,
            tc.tile_pool(name="gdata", bufs=5) as gpool,
            tc.tile_pool(name="kdata", bufs=4) as kpool,
            tc.tile_pool(name="accs", bufs=8) as apool,
            tc.tile_pool(name="psum", bufs=4, space="PSUM") as ppool,
            tc.tile_pool(name="psig", bufs=1, space="PSUM") as pspool,
        ):
            # vrow lands in partition 0 of the broadcast destination itself;
            # per-chunk PE ones-matmuls + ScalarE PSUM-drains then overwrite
            # partitions 0..127 with the replicated row (value-identical on
            # partition 0). The ones lhsT lives in the untouched tail.
            vb_sb = cpool.tile([P, VHLEN], f32)
            nc.sync.dma_start(out=vb_sb[0:1, :], in_=vh[:, :])

            # The PSUM drain runs on DVE itself (idle until its first STT):
            # copies interleave with tile-0's compute in engine program
            # order, so the broadcast never paces the stream the way a
            # PE<->ScalarE sem ping-pong does. Chunks are emitted lazily,
            # just before the first STT that reads them.
            bc_next = [0]

            def emit_broadcast_upto(chunk_end):
                while bc_next[0] < chunk_end:
                    c = bc_next[0]
                    bc_next[0] += 1
                    ps = ppool.tile([P, BC], f32, tag="bc", name=f"ps_{c}")
                    nc.tensor.matmul(ps[:, :], vb_sb[0:1, MPAD:MPAD + P],
                                     vb_sb[0:1, c * BC:(c + 1) * BC],
                                     start=True, stop=True)
                    nc.vector.tensor_copy(
                        out=vb_sb[:, c * BC:(c + 1) * BC], in_=ps[:, :])

            # Primers (see loop below) stage DVE's vector clock against the
            # ScalarE broadcast copies: each tensor_copy waits on the copy
            # covering the end of a vb range, after which the STTs reading
            # that range carry only their own DMA wait. Each primer gets a
            # private slot — sharing one tile would chain same-engine WAW
            # deps and push an instruction to two waits.
            def primer_read(col):
                pt = apool.tile([P, 2], f32, tag="acc")
                nc.vector.tensor_copy(out=pt[:1, 0:1],
                                      in_=vb_sb[:1, col:col + 1])

            out_all = cpool.tile([P, NT], f32)
            nc.vector.memset(out_all[:, :], 0.0)

            for t in range(NT):
                p = min(P, N - t * P)
                i0 = t * P
                # Fused multiply+reduce per streamed tile: out = (data * 1.0)
                # * vb, accum = row-sum. Column 1 of each private acc tile
                # absorbs the dead full-size out via a stride-0 broadcast AP.
                # Tile 0 streams in 3000-wide quarters so the first STT only
                # has to wait for the first six broadcast chunks.
                if t == 0:
                    ranges = [(q * GC // 2, (q + 1) * GC // 2) for q in range(4)]
                else:
                    ranges = [(0, GC), (GC, MG)]
                accs = []
                for lo, hi in ranges:
                    gt = gpool.tile([P, hi - lo], f32, tag="g")
                    nc.sync.dma_start(out=gt[:p, :], in_=g[i0:i0 + p, lo:hi])
                    if t == 0:
                        emit_broadcast_upto((hi + BC - 1) // BC)
                        primer_read(hi - 1)
                    acc = apool.tile([P, 2], f32, tag="acc")
                    nc.vector.scalar_tensor_tensor(
                        out=acc[:p, 1:2].broadcast_to((p, hi - lo)),
                        in0=gt[:p, :],
                        scalar=1.0,
                        in1=vb_sb[:p, lo:hi],
                        op0=mult,
                        op1=mult,
                        accum_out=acc[:p, 0:1],
                    )
                    accs.append(acc)
                kt = kpool.tile([P, N], f32, tag="k")
                nc.sync.dma_start(out=kt[:p, :], in_=k[i0:i0 + p, :])
                if t == 0:
                    emit_broadcast_upto(NBC)
                    primer_read(MTOT - 1)
                acc = apool.tile([P, 2], f32, tag="acc")
                nc.vector.scalar_tensor_tensor(
                    out=acc[:p, 1:2].broadcast_to((p, N)),
                    in0=kt[:p, :],
                    scalar=1.0,
                    in1=vb_sb[:p, MG:MTOT],
                    op0=mult,
                    op1=mult,
                    accum_out=acc[:p, 0:1],
                )
                accs.append(acc)
                # Reduce the 3 (or 5) partial sums into out[:, t], three per
                # STT (two tensor operands plus the per-partition scalar AP).
                sums = [a[:p, 0:1] for a in accs]
                while len(sums) > 1:
                    take, sums = sums[:3], sums[3:]
                    assert len(take) == 3  # counts here are 3 or 5 → 3
                    final = not sums
                    if final:
                        dst = out_all[:p, t:t + 1]
                    else:
                        tmp_acc = apool.tile([P, 2], f32, tag="acc",
                                             name=f"tmp_acc_{t}")
                        dst = tmp_acc[:p, 0:1]
                    nc.vector.scalar_tensor_tensor(
                        out=dst,
                        in0=take[0],
                        scalar=take[1],
                        in1=take[2],
                        op0=add,
                        op1=add,
                    )
                    if not final:
                        sums.insert(0, dst)
            nc.sync.dma_start(out=o[:, :], in_=out_all[:, :])

            # Sigma = ltaug.T @ ltaug on the TensorEngine. Emitted after the
            # loop so its DVE PSUM-drain doesn't stall the streaming STTs at
            # kernel start.
            lt_sb = cpool.tile([2 * D, D], f32)
            nc.sync.dma_start(out=lt_sb[:, :], in_=lt[:, :])
            sig_ps = pspool.tile([D, D], f32)
            nc.tensor.matmul(sig_ps[:, :], lt_sb[:, :], lt_sb[:, :],
                             start=True, stop=True)
            sig_sb = cpool.tile([D, D], f32)
            nc.vector.tensor_copy(out=sig_sb[:, :], in_=sig_ps[:, :])
            nc.sync.dma_start(out=sig[:, :], in_=sig_sb[:, :])
    nc.finalize()
    return nc


def _get_nc():
    global _COMPILED
    if _COMPILED is None:
        _COMPILED = _build()
    return _COMPILED


def run(inputs, trace=False):
    """Run the SPMD kernel; returns ((output, Sigma), BassKernelResults)."""
    from concourse.bass_utils import run_bass_kernel_spmd

    alpha = np.ascontiguousarray(np.asarray(inputs["alpha"], dtype=np.float32))
    beta = np.ascontiguousarray(np.asarray(inputs["beta"], dtype=np.float32))
    L = np.ascontiguousarray(np.asarray(inputs["L"], dtype=np.float32))
    K = np.ascontiguousarray(np.asarray(inputs["K"], dtype=np.float32))
    grad_K2 = np.ascontiguousarray(np.asarray(inputs["grad_K2"], dtype=np.float32))

    ltaug = np.concatenate(
        [L.T, 1e-3 * np.eye(D, dtype=np.float32)], axis=0
    ).astype(np.float32)

    in_maps = []
    for j in range(NCORES):
        vh = np.zeros((1, VHLEN), dtype=np.float32)
        vh[0, :MG] = np.ascontiguousarray(beta[j].T).reshape(-1)
        vh[0, MG:MTOT] = alpha[j]
        vh[0, MPAD:] = 1.0
        in_maps.append({
            "g": grad_K2[j].reshape(N, MG),
            "k": K[j],
            "vh": vh,
            "lt": ltaug,
        })

    nc = _get_nc()
    res = run_bass_kernel_spmd(nc, in_maps, core_ids=list(range(NCORES)),
                               trace=trace)
    output = np.empty((N, D), dtype=np.float32)
    for j in range(NCORES):
        col = res.results[j]["o"]          # [128, 12]
        output[:, j] = col.T.reshape(-1)[:N]
    Sigma = res.results[0]["sig"]
    return (output, Sigma), res


def kernel(**inputs):
    out, _ = run(inputs)
    return out


# revision 52
# speedup vs baseline: 1.0527x; 1.0527x over previous
"""Trainium2 Bass kernel for the ADMG RKHS-DAGMA gradient contraction.

Reference computation (D=8 variables, N=1500 observations):
    output[i, j] = sum_l alpha[j, l] * K[j, i, l]
                 + sum_{a, l} beta[j, a, l] * grad_K2[j, i, l, a]     [N, D]
    Sigma = L @ L.T + 1e-6 * I                                        [D, D]

Sharding: variable-parallel over the leading d axis — core j owns K[j]
(9 MB) and grad_K2[j] (72 MB) and produces output column j. No
collectives are needed; columns are gathered on the host.

Per-core kernel: a weighted row-sum. With G = grad_K2[j] viewed as
[N, L*A] = [1500, 12000] and vrow = concat(beta[j].T.flat, alpha[j])
(13500 f32), the output column is
    out[i] = sum_m [G | K][i, m] * vrow[m].
vrow is broadcast across the 128 partitions on-chip (PE ones-matmul into
PSUM, ScalarE drains to SBUF), then every streamed [128, C] tile goes
through one fused DVE scalar_tensor_tensor (multiply + free-axis
accumulate), so each element passes the VectorEngine exactly once
(~170us) and the kernel stays DMA-bound (~82 MB/core at ~360 GB/s).

TRN2 toolchain constraints baked in here:
  - every instruction except EventSemaphore encodes exactly ONE
    semaphore wait; Tile emits inline waits, so the kernel is structured
    (private acc tiles, clock-priming reads) to never need two;
  - the raw-ISA tensor_tensor_reduce encoding crashes the exec unit on
    this runtime; InstTensorScalarPtr (scalar_tensor_tensor with
    accum_out) performs the same fused multiply+reduce;
  - Sigma's +1e-6*I rides inside the matmul via an augmented-contraction
    operand [L.T ; 1e-3*I] so no PSUM+SBUF add is needed.
"""

import numpy as np

D = 8
N = 1500
NCORES = 8
MG = D * N            # 12000: grad_K2 inner (l, a) length
MTOT = MG + N         # 13500: plus K's l axis
P = 128
NT = (N + P - 1) // P  # 12 i-tiles (last one 92 rows)
BC = 512               # broadcast chunk: one fp32 matmul free dim / PSUM bank
NBC = (MTOT + BC - 1) // BC          # 27 chunks
MPAD = NBC * BC                      # 13824
VHLEN = MPAD + P                     # vrow (padded) + ones-vector tail
GC = MG // 2           # 6000-wide g chunks (two per i-tile)


_COMPILED = None


def _build():
    from concourse import bacc, mybir
    from concourse.tile import TileContext

    f32 = mybir.dt.float32
    nc = bacc.Bacc()

    g = nc.declare_dram_parameter("g", [N, MG], f32, isOutput=False)
    k = nc.declare_dram_parameter("k", [N, N], f32, isOutput=False)
    # One row: vrow zero-padded to MPAD, then 128 ones (the PE-broadcast
    # lhsT) — a single row so both matmul operands sit at base partition 0
    # and depend on one DMA.
    vh = nc.declare_dram_parameter("vh", [1, VHLEN], f32, isOutput=False)
    # Rows 0..D-1 = L.T, rows D..2D-1 = 1e-3 * I, so that one matmul gives
    # ltaug.T @ ltaug = L @ L.T + 1e-6 * I.
    lt = nc.declare_dram_parameter("lt", [2 * D, D], f32, isOutput=False)
    o = nc.declare_dram_parameter("o", [P, NT], f32, isOutput=True)
    sig = nc.declare_dram_parameter("sig", [D, D], f32, isOutput=True)

    mult = mybir.AluOpType.mult
    add = mybir.AluOpType.add

    with TileContext(nc) as tc:
        with (
            tc.tile_pool(name="const", bufs=1) as cpool,
            tc.tile_pool(name="gdata", bufs=5) as gpool,
            tc.tile_pool(name="kdata", bufs=4) as kpool,
            tc.tile_pool(name="accs", bufs=8) as apool,
            tc.tile_pool(name="psum", bufs=4, space="PSUM") as ppool,
            tc.tile_pool(name="psig", bufs=1, space="PSUM") as pspool,
        ):
            # vrow lands in partition 0 of the broadcast destination itself;
            # per-chunk PE ones-matmuls + ScalarE PSUM-drains then overwrite
            # partitions 0..127 with the replicated row (value-identical on
            # partition 0). The ones lhsT lives in the untouched tail.
            vb_sb = cpool.tile([P, VHLEN], f32)
            nc.sync.dma_start(out=vb_sb[0:1, :], in_=vh[:, :])

            # The PSUM drain runs on DVE itself (idle until its first STT):
            # copies interleave with tile-0's compute in engine program
            # order, so the broadcast never paces the stream the way a
            # PE<->ScalarE sem ping-pong does. Chunks are emitted lazily,
            # just before the first STT that reads them.
            bc_next = [0]

            def emit_broadcast_upto(chunk_end):
                while bc_next[0] < chunk_end:
                    c = bc_next[0]
                    bc_next[0] += 1
                    ps = ppool.tile([P, BC], f32, tag="bc", name=f"ps_{c}")
                    nc.tensor.matmul(ps[:, :], vb_sb[0:1, MPAD:MPAD + P],
                                     vb_sb[0:1, c * BC:(c + 1) * BC],
                                     start=True, stop=True)
                    nc.vector.tensor_copy(
                        out=vb_sb[:, c * BC:(c + 1) * BC], in_=ps[:, :])

            # Primers (see loop below) stage DVE's vector clock against the
            # ScalarE broadcast copies: each tensor_copy waits on the copy
            # covering the end of a vb range, after which the STTs reading
            # that range carry only their own DMA wait. Each primer gets a
            # private slot — sharing one tile would chain same-engine WAW
            # deps and push an instruction to two waits.
            def primer_read(col):
                pt = apool.tile([P, 2], f32, tag="acc")
                nc.vector.tensor_copy(out=pt[:1, 0:1],
                                      in_=vb_sb[:1, col:col + 1])

            out_all = cpool.tile([P, NT], f32)
            nc.vector.memset(out_all[:, :], 0.0)

            for t in range(NT):
                p = min(P, N - t * P)
                i0 = t * P
                # Fused multiply+reduce per streamed tile: out = (data * 1.0)
                # * vb, accum = row-sum. Column 1 of each private acc tile
                # absorbs the dead full-size out via a stride-0 broadcast AP.
                # Tile 0 streams in 3000-wide quarters so the first STT only
                # has to wait for the first six broadcast chunks.
                if t == 0:
                    ranges = [(q * GC // 2, (q + 1) * GC // 2) for q in range(4)]
                else:
                    ranges = [(0, GC), (GC, MG)]
                accs = []
                for lo, hi in ranges:
                    gt = gpool.tile([P, hi - lo], f32, tag="g")
                    nc.sync.dma_start(out=gt[:p, :], in_=g[i0:i0 + p, lo:hi])
                    if t == 0:
                        emit_broadcast_upto((hi + BC - 1) // BC)
                        primer_read(hi - 1)
                    acc = apool.tile([P, 2], f32, tag="acc")
                    nc.vector.scalar_tensor_tensor(
                        out=acc[:p, 1:2].broadcast_to((p, hi - lo)),
                        in0=gt[:p, :],
                        scalar=1.0,
                        in1=vb_sb[:p, lo:hi],
                        op0=mult,
                        op1=mult,
                        accum_out=acc[:p, 0:1],
                    )
                    accs.append(acc)
                kt = kpool.tile([P, N], f32, tag="k")
                nc.scalar.dma_start(out=kt[:p, :], in_=k[i0:i0 + p, :])
                if t == 0:
                    emit_broadcast_upto(NBC)
                    primer_read(MTOT - 1)
                acc = apool.tile([P, 2], f32, tag="acc")
                nc.vector.scalar_tensor_tensor(
                    out=acc[:p, 1:2].broadcast_to((p, N)),
                    in0=kt[:p, :],
                    scalar=1.0,
                    in1=vb_sb[:p, MG:MTOT],
                    op0=mult,
                    op1=mult,
                    accum_out=acc[:p, 0:1],
                )
                accs.append(acc)
                # Reduce the 3 (or 5) partial sums into out[:, t], three per
                # STT (two tensor operands plus the per-partition scalar AP).
                sums = [a[:p, 0:1] for a in accs]
                while len(sums) > 1:
                    take, sums = sums[:3], sums[3:]
                    assert len(take) == 3  # counts here are 3 or 5 → 3
                    final = not sums
                    if final:
                        dst = out_all[:p, t:t + 1]
                    else:
                        tmp_acc = apool.tile([P, 2], f32, tag="acc",
                                             name=f"tmp_acc_{t}")
                        dst = tmp_acc[:p, 0:1]
                    nc.vector.scalar_tensor_tensor(
                        out=dst,
                        in0=take[0],
                        scalar=take[1],
                        in1=take[2],
                        op0=add,
                        op1=add,
                    )
                    if not final:
                        sums.insert(0, dst)
            nc.sync.dma_start(out=o[:, :], in_=out_all[:, :])

            # Sigma = ltaug.T @ ltaug on the TensorEngine. Emitted after the
            # loop so its DVE PSUM-drain doesn't stall the streaming STTs at
            # kernel start.
            lt_sb = cpool.tile([2 * D, D], f32)
            nc.sync.dma_start(out=lt_sb[:, :], in_=lt[:, :])
            sig_ps = pspool.tile([D, D], f32)
            nc.tensor.matmul(sig_ps[:, :], lt_sb[:, :], lt_sb[:, :],
                             start=True, stop=True)
            sig_sb = cpool.tile([D, D], f32)
            nc.vector.tensor_copy(out=sig_sb[:, :], in_=sig_ps[:, :])
            nc.sync.dma_start(out=sig[:, :], in_=sig_sb[:, :])
    nc.finalize()
    return nc


def _get_nc():
    global _COMPILED
    if _COMPILED is None:
        _COMPILED = _build()
    return _COMPILED


def run(inputs, trace=False):
    """Run the SPMD kernel; returns ((output, Sigma), BassKernelResults)."""
    from concourse.bass_utils import run_bass_kernel_spmd

    alpha = np.ascontiguousarray(np.asarray(inputs["alpha"], dtype=np.float32))
    beta = np.ascontiguousarray(np.asarray(inputs["beta"], dtype=np.float32))
    L = np.ascontiguousarray(np.asarray(inputs["L"], dtype=np.float32))
    K = np.ascontiguousarray(np.asarray(inputs["K"], dtype=np.float32))
    grad_K2 = np.ascontiguousarray(np.asarray(inputs["grad_K2"], dtype=np.float32))

    ltaug = np.concatenate(
        [L.T, 1e-3 * np.eye(D, dtype=np.float32)], axis=0
    ).astype(np.float32)

    in_maps = []
    for j in range(NCORES):
        vh = np.zeros((1, VHLEN), dtype=np.float32)
        vh[0, :MG] = np.ascontiguousarray(beta[j].T).reshape(-1)
        vh[0, MG:MTOT] = alpha[j]
        vh[0, MPAD:] = 1.0
        in_maps.append({
            "g": grad_K2[j].reshape(N, MG),
            "k": K[j],
            "vh": vh,
            "lt": ltaug,
        })

    nc = _get_nc()
    res = run_bass_kernel_spmd(nc, in_maps, core_ids=list(range(NCORES)),
                               trace=trace)
    output = np.empty((N, D), dtype=np.float32)
    for j in range(NCORES):
        col = res.results[j]["o"]          # [128, 12]
        output[:, j] = col.T.reshape(-1)[:N]
    Sigma = res.results[0]["sig"]
    return (output, Sigma), res


def kernel(**inputs):
    out, _ = run(inputs)
    return out


# revision 54
# speedup vs baseline: 1.1872x; 1.1278x over previous
"""Trainium2 Bass kernel for the ADMG RKHS-DAGMA gradient contraction.

Reference computation (D=8 variables, N=1500 observations):
    output[i, j] = sum_l alpha[j, l] * K[j, i, l]
                 + sum_{a, l} beta[j, a, l] * grad_K2[j, i, l, a]     [N, D]
    Sigma = L @ L.T + 1e-6 * I                                        [D, D]

Sharding: variable-parallel over the leading d axis — core j owns K[j]
(9 MB) and grad_K2[j] (72 MB) and produces output column j. No
collectives are needed; columns are gathered on the host.

Per-core kernel: a weighted row-sum. With G = grad_K2[j] viewed as
[N, L*A] = [1500, 12000] and vrow = concat(beta[j].T.flat, alpha[j])
(13500 f32), the output column is
    out[i] = sum_m [G | K][i, m] * vrow[m].
vrow is broadcast across the 128 partitions on-chip (PE ones-matmul into
PSUM, ScalarE drains to SBUF), then every streamed [128, C] tile goes
through one fused DVE scalar_tensor_tensor (multiply + free-axis
accumulate), so each element passes the VectorEngine exactly once
(~170us) and the kernel stays DMA-bound (~82 MB/core at ~360 GB/s).

TRN2 toolchain constraints baked in here:
  - every instruction except EventSemaphore encodes exactly ONE
    semaphore wait; Tile emits inline waits, so the kernel is structured
    (private acc tiles, clock-priming reads) to never need two;
  - the raw-ISA tensor_tensor_reduce encoding crashes the exec unit on
    this runtime; InstTensorScalarPtr (scalar_tensor_tensor with
    accum_out) performs the same fused multiply+reduce;
  - Sigma's +1e-6*I rides inside the matmul via an augmented-contraction
    operand [L.T ; 1e-3*I] so no PSUM+SBUF add is needed.
"""

import numpy as np

D = 8
N = 1500
NCORES = 8
MG = D * N            # 12000: grad_K2 inner (l, a) length
MTOT = MG + N         # 13500: plus K's l axis
P = 128
NT = (N + P - 1) // P  # 12 i-tiles (last one 92 rows)
BC = 512               # broadcast chunk: one fp32 matmul free dim / PSUM bank
NBC = (MTOT + BC - 1) // BC          # 27 chunks
MPAD = NBC * BC                      # 13824
VHLEN = MPAD + P                     # vrow (padded) + ones-vector tail
GC = MG // 2           # 6000-wide g chunks (two per i-tile)


_COMPILED = None


def _build():
    from concourse import bacc, mybir
    from concourse.tile import TileContext

    f32 = mybir.dt.float32
    nc = bacc.Bacc()

    g = nc.declare_dram_parameter("g", [N, MG], f32, isOutput=False)
    k = nc.declare_dram_parameter("k", [N, N], f32, isOutput=False)
    # One row: vrow zero-padded to MPAD, then 128 ones (the PE-broadcast
    # lhsT) — a single row so both matmul operands sit at base partition 0
    # and depend on one DMA.
    vh = nc.declare_dram_parameter("vh", [1, VHLEN], f32, isOutput=False)
    # Rows 0..D-1 = L.T, rows D..2D-1 = 1e-3 * I, so that one matmul gives
    # ltaug.T @ ltaug = L @ L.T + 1e-6 * I.
    lt = nc.declare_dram_parameter("lt", [2 * D, D], f32, isOutput=False)
    o = nc.declare_dram_parameter("o", [P, NT], f32, isOutput=True)
    sig = nc.declare_dram_parameter("sig", [D, D], f32, isOutput=True)

    mult = mybir.AluOpType.mult
    add = mybir.AluOpType.add

    with TileContext(nc) as tc:
        with (
            tc.tile_pool(name="const", bufs=1) as cpool,
            tc.tile_pool(name="gdata", bufs=10) as gpool,
            tc.tile_pool(name="kdata", bufs=4) as kpool,
            tc.tile_pool(name="accs", bufs=8) as apool,
            tc.tile_pool(name="psum", bufs=4, space="PSUM") as ppool,
            tc.tile_pool(name="psig", bufs=1, space="PSUM") as pspool,
        ):
            # vrow lands in partition 0 of the broadcast destination itself;
            # per-chunk PE ones-matmuls + ScalarE PSUM-drains then overwrite
            # partitions 0..127 with the replicated row (value-identical on
            # partition 0). The ones lhsT lives in the untouched tail.
            vb_sb = cpool.tile([P, VHLEN], f32)
            nc.sync.dma_start(out=vb_sb[0:1, :], in_=vh[:, :])

            # The PSUM drain runs on DVE itself (idle until its first STT):
            # copies interleave with tile-0's compute in engine program
            # order, so the broadcast never paces the stream the way a
            # PE<->ScalarE sem ping-pong does. Chunks are emitted lazily,
            # just before the first STT that reads them.
            bc_next = [0]

            def emit_broadcast_upto(chunk_end):
                while bc_next[0] < chunk_end:
                    c = bc_next[0]
                    bc_next[0] += 1
                    ps = ppool.tile([P, BC], f32, tag="bc", name=f"ps_{c}")
                    nc.tensor.matmul(ps[:, :], vb_sb[0:1, MPAD:MPAD + P],
                                     vb_sb[0:1, c * BC:(c + 1) * BC],
                                     start=True, stop=True)
                    nc.vector.tensor_copy(
                        out=vb_sb[:, c * BC:(c + 1) * BC], in_=ps[:, :])

            # Primers (see loop below) stage DVE's vector clock against the
            # ScalarE broadcast copies: each tensor_copy waits on the copy
            # covering the end of a vb range, after which the STTs reading
            # that range carry only their own DMA wait. Each primer gets a
            # private slot — sharing one tile would chain same-engine WAW
            # deps and push an instruction to two waits.
            def primer_read(col):
                pt = apool.tile([P, 2], f32, tag="acc")
                nc.vector.tensor_copy(out=pt[:1, 0:1],
                                      in_=vb_sb[:1, col:col + 1])

            out_all = cpool.tile([P, NT], f32)
            nc.vector.memset(out_all[:, :], 0.0)

            for t in range(NT):
                p = min(P, N - t * P)
                i0 = t * P
                # Fused multiply+reduce per streamed tile: out = (data * 1.0)
                # * vb, accum = row-sum. Column 1 of each private acc tile
                # absorbs the dead full-size out via a stride-0 broadcast AP.
                # Tile 0 streams in 3000-wide quarters so the first STT only
                # has to wait for the first six broadcast chunks.
                ranges = [(q * GC // 2, (q + 1) * GC // 2) for q in range(4)]
                accs = []
                for lo, hi in ranges:
                    gt = gpool.tile([P, hi - lo], f32, tag="g")
                    nc.sync.dma_start(out=gt[:p, :], in_=g[i0:i0 + p, lo:hi])
                    if t == 0:
                        emit_broadcast_upto((hi + BC - 1) // BC)
                        primer_read(hi - 1)
                    acc = apool.tile([P, 2], f32, tag="acc")
                    nc.vector.scalar_tensor_tensor(
                        out=acc[:p, 1:2].broadcast_to((p, hi - lo)),
                        in0=gt[:p, :],
                        scalar=1.0,
                        in1=vb_sb[:p, lo:hi],
                        op0=mult,
                        op1=mult,
                        accum_out=acc[:p, 0:1],
                    )
                    accs.append(acc)
                kt = kpool.tile([P, N], f32, tag="k")
                nc.sync.dma_start(out=kt[:p, :], in_=k[i0:i0 + p, :])
                if t == 0:
                    emit_broadcast_upto(NBC)
                    primer_read(MTOT - 1)
                acc = apool.tile([P, 2], f32, tag="acc")
                nc.vector.scalar_tensor_tensor(
                    out=acc[:p, 1:2].broadcast_to((p, N)),
                    in0=kt[:p, :],
                    scalar=1.0,
                    in1=vb_sb[:p, MG:MTOT],
                    op0=mult,
                    op1=mult,
                    accum_out=acc[:p, 0:1],
                )
                accs.append(acc)
                # Reduce the 3 (or 5) partial sums into out[:, t], three per
                # STT (two tensor operands plus the per-partition scalar AP).
                sums = [a[:p, 0:1] for a in accs]
                while len(sums) > 1:
                    take, sums = sums[:3], sums[3:]
                    assert len(take) == 3  # counts here are 3 or 5 → 3
                    final = not sums
                    if final:
                        dst = out_all[:p, t:t + 1]
                    else:
                        tmp_acc = apool.tile([P, 2], f32, tag="acc",
                                             name=f"tmp_acc_{t}")
                        dst = tmp_acc[:p, 0:1]
                    nc.vector.scalar_tensor_tensor(
                        out=dst,
                        in0=take[0],
                        scalar=take[1],
                        in1=take[2],
                        op0=add,
                        op1=add,
                    )
                    if not final:
                        sums.insert(0, dst)
            nc.sync.dma_start(out=o[:, :], in_=out_all[:, :])

            # Sigma = ltaug.T @ ltaug on the TensorEngine. Emitted after the
            # loop so its DVE PSUM-drain doesn't stall the streaming STTs at
            # kernel start.
            lt_sb = cpool.tile([2 * D, D], f32)
            nc.sync.dma_start(out=lt_sb[:, :], in_=lt[:, :])
            sig_ps = pspool.tile([D, D], f32)
            nc.tensor.matmul(sig_ps[:, :], lt_sb[:, :], lt_sb[:, :],
                             start=True, stop=True)
            sig_sb = cpool.tile([D, D], f32)
            nc.vector.tensor_copy(out=sig_sb[:, :], in_=sig_ps[:, :])
            nc.sync.dma_start(out=sig[:, :], in_=sig_sb[:, :])
    nc.finalize()
    return nc


def _get_nc():
    global _COMPILED
    if _COMPILED is None:
        _COMPILED = _build()
    return _COMPILED


def run(inputs, trace=False):
    """Run the SPMD kernel; returns ((output, Sigma), BassKernelResults)."""
    from concourse.bass_utils import run_bass_kernel_spmd

    alpha = np.ascontiguousarray(np.asarray(inputs["alpha"], dtype=np.float32))
    beta = np.ascontiguousarray(np.asarray(inputs["beta"], dtype=np.float32))
    L = np.ascontiguousarray(np.asarray(inputs["L"], dtype=np.float32))
    K = np.ascontiguousarray(np.asarray(inputs["K"], dtype=np.float32))
    grad_K2 = np.ascontiguousarray(np.asarray(inputs["grad_K2"], dtype=np.float32))

    ltaug = np.concatenate(
        [L.T, 1e-3 * np.eye(D, dtype=np.float32)], axis=0
    ).astype(np.float32)

    in_maps = []
    for j in range(NCORES):
        vh = np.zeros((1, VHLEN), dtype=np.float32)
        vh[0, :MG] = np.ascontiguousarray(beta[j].T).reshape(-1)
        vh[0, MG:MTOT] = alpha[j]
        vh[0, MPAD:] = 1.0
        in_maps.append({
            "g": grad_K2[j].reshape(N, MG),
            "k": K[j],
            "vh": vh,
            "lt": ltaug,
        })

    nc = _get_nc()
    res = run_bass_kernel_spmd(nc, in_maps, core_ids=list(range(NCORES)),
                               trace=trace)
    output = np.empty((N, D), dtype=np.float32)
    for j in range(NCORES):
        col = res.results[j]["o"]          # [128, 12]
        output[:, j] = col.T.reshape(-1)[:N]
    Sigma = res.results[0]["sig"]
    return (output, Sigma), res


def kernel(**inputs):
    out, _ = run(inputs)
    return out
